# revision 34
# baseline (speedup 1.0000x reference)
"""Batch multi-head graph attention (GAT) kernel for 8 Trainium2 NeuronCores.

Reference computation (per batch b, head g):
    hp   = h[b] @ w[g]                        # [N, O]
    t    = tanh(hp)
    src  = t @ a_src[g];  dst = t @ a_dst[g]  # [N]
    s    = leaky_relu(src[q] + dst[m], 0.2)   # [N(q), N(m)]
    e    = exp(s) masked by adj[b][q, m]
    out  = (e @ hp) / rowsum(e) + bias

Device strategy (per core; core c -> b = c//2, heads = (2*(c%2), 2*(c%2)+1)):
  * scores are built TRANSPOSED: tiles [128 m(keys), 2048 q(queries)] so the
    output matmul out^T[o, q] = sum_m hp[m, o] * e[m, q] streams on PE with
    the contraction dim on partitions.
  * exp factorization with query-scale cancellation and a 1/64 damping:
        exp(leaky(s))/64 = exp(0.2 src)[q] * max(c_b[q] * edst[m], e02[m])
    with c_b = exp(0.8 src - ln64)[q], edst = exp(dst)[m],
    e02 = exp(0.2 dst - ln64)[m].  The common per-query factor exp(0.2 src)
    cancels in the final division and is never computed; the 1/64 keeps all
    score values strictly below 255 and also cancels.
  * the adjacency mask arrives as bf16 {0, 1} (host-prepared, no cast
    DMA so all dispatch stays on HWDGE) and is applied by multiply.
  * per chunk the scores are ONE dual-op tensor_scalar (DVE 4x mode) plus
    one mask multiply (DVE 2x mode; the first 3 chunks of each pair run
    their mask on GPSIMD instead).  The matmul accumulation order
    interleaves Pool-masked chunks by expected readiness so the slow Pool
    ops never gate the accumulation tail.
  * h and w are cast to bf16 on the host (B1 matmuls at 1 cycle/row, half
    the h DMA); c_b is exp'ed row-wise FIRST (16x128 elements) and
    broadcast with a bf16 selector matmul (1 cycle/row).  B4 fuses the
    softmax division into the transposed-psum drain (q lands on
    partitions, so 1/den is a per-partition ACT scale).
  * softmax denominator rides as a ones-column in the matmul lhsT; bias is
    folded into hp (softmax weights sum to exactly 1).  Final transpose
    back to [q, o] via PE transpose; normalization runs on ACT.
"""

import os
import sys

for _p in ("/opt/trn_rl_repo",):
    if _p not in sys.path and os.path.isdir(_p):
        sys.path.insert(0, _p)

from contextlib import ExitStack

import numpy as np
import ml_dtypes

import concourse.bass as bass
import concourse.tile as tile
from concourse import bacc, mybir
from concourse.bass_utils import run_bass_kernel_spmd
from concourse.tile_rust import add_dep_helper

F32 = mybir.dt.float32
BF16 = mybir.dt.bfloat16
U8 = mybir.dt.uint8
AF = mybir.ActivationFunctionType
OP = mybir.AluOpType
AX = mybir.AxisListType

N = 2048          # nodes
F = 256           # input features
O = 64            # output features
W = O + 1         # hp columns + ones column
P = 128           # partitions
NCH = N // P      # 16 m-chunks per pair
NEG_SLOPE = 0.2
N_CORES = 8
NADJ = 8          # adj arrives as NADJ independent slabs for overlap
LN_DAMP = float(np.log(64.0))   # score damping; cancels in the division

# m-chunks whose min-mask runs on GPSIMD instead of DVE (load balance);
# shared by both pairs.  U8_SLABS are the slabs (2 chunks each) all of
# whose chunks are Pool-masked: they skip the bf16 cast DMA.
POOL_MC = (0, 1, 2)               # mask-mult on GPSIMD (only mult/add ucode)
DVE_MC = tuple(mc for mc in range(16) if mc not in POOL_MC)
# matmul accumulation order, interleaved by expected u-readiness so the
# last chunk in the sequence is never waiting on a slow Pool mask
B3_SEQ = (3, 4, 5, 6, 7, 0, 8, 9, 10, 1, 11, 12, 2, 13, 14, 15)

# when the bias input is all-zero (it is for this problem), hp needs no add;
# set per-build from the actual bias values in kernel()
ZERO_BIAS = True


class PairCtx:
    pass


def _emit_b1(nc, cpool, pspool, consts, hT, pair):
    """allocate per-pair hp/t_cat/projection tiles (filled in stage A)."""
    px = PairCtx()
    px.hp_big = cpool.tile([P, NCH * W], BF16, tag="hp_big", bufs=2, name=f"hp{pair}")
    px.t_cat = cpool.tile([P, NCH * O], F32, tag="t_cat", bufs=2, name=f"tcat{pair}")
    px.prod = cpool.tile([P, NCH * O], F32, tag="prod", bufs=2, name=f"prod{pair}")
    px.dprod = cpool.tile([P, NCH * O], F32, tag="dprod", bufs=2, name=f"dprod{pair}")
    px.src_col = cpool.tile([P, NCH], F32, tag="src_col", bufs=2, name=f"srcc{pair}")
    px.dst_col = cpool.tile([P, NCH], F32, tag="dst_col", bufs=2, name=f"dstc{pair}")
    return px


GB1 = 16  # all chunks in ONE [128,1024] psum tile per pair: single tanh /
          # drain / projection chain minimizes cross-engine sync hops


def _emit_b1_group(nc, pspool, consts, hT, pair, px, g, do_reduce):
    ident_sb, sel_sb, w_sb, asrc_sb, adst_sb, bias_sb = consts
    psum_hp = pspool.tile([P, GB1 * O], F32, tag="hp", bufs=2,
                          name=f"pshp{pair}_{g}")
    for i in range(GB1):
        mc = g * GB1 + i
        for fc in range(2):
            nc.tensor.matmul(
                psum_hp[:, i * O:(i + 1) * O],
                hT[fc][:, mc * P:(mc + 1) * P],
                w_sb[:, (2 * pair + fc) * O:(2 * pair + fc + 1) * O],
                start=(fc == 0),
                stop=(fc == 1),
                skip_group_check=True,
            )
    hp_dst = px.hp_big.rearrange("p (c k) -> p c k", k=W)[:, g * GB1:(g + 1) * GB1, :O]
    ps_v = psum_hp.rearrange("p (c k) -> p c k", k=O)
    if not ZERO_BIAS:
        nc.vector.tensor_tensor(ps_v, ps_v, bias_sb.rearrange(
            "p (c k) -> p c k", c=1).broadcast_to((P, GB1, O)), OP.add)
    if pair == 0:
        # pair 0's psum drain rides the idle front DVE; pair 1's runs on
        # ACT so it never head-of-line blocks pair 0's B3 score ops.
        nc.vector.tensor_copy(hp_dst, ps_v)
    else:
        nc.scalar.copy(hp_dst, ps_v)
    nc.scalar.activation(
        px.t_cat[:, g * GB1 * O:(g + 1) * GB1 * O], psum_hp[:], AF.Tanh
    )
    # src/dst projection multiplies pipeline right behind each tanh group
    # on Pool (a-vectors repeated via stride-0 broadcast APs); the DVE
    # X-reductions run here only for pair 0 (front DVE is idle) -- pair 1's
    # are deferred to B2 so they sit behind B3p0 in the DVE queue.
    gs = slice(g * GB1, (g + 1) * GB1)
    t_v = px.t_cat.rearrange("p (c k) -> p c k", k=O)[:, gs, :]
    prod_sl = slice(g * GB1, (g + 1) * GB1) if pair == 0 else gs
    prod_v = px.prod.rearrange("p (c k) -> p c k", k=O)[:, gs, :]
    asrc_b = asrc_sb[:, pair * O:(pair + 1) * O] \
        .rearrange("p (c o) -> p c o", c=1).broadcast_to((P, GB1, O))
    adst_b = adst_sb[:, pair * O:(pair + 1) * O] \
        .rearrange("p (c o) -> p c o", c=1).broadcast_to((P, GB1, O))
    # pair 0's whole projection chain rides the idle front DVE: cross-
    # engine semaphore waits resolve at coarse tick boundaries, so keeping
    # the tanh -> prod -> reduce -> c_b chain off Pool saves ~6us of
    # latency.  pair 1's prods go to Pool (their consumers run much later).
    mul_eng = nc.vector if pair == 0 else nc.gpsimd
    mul_eng.tensor_tensor(prod_v, t_v, asrc_b, OP.mult)
    if do_reduce:
        nc.vector.tensor_reduce(px.src_col[:, gs], prod_v, AX.X, OP.add)
    dprod_v = px.dprod.rearrange("p (c k) -> p c k", k=O)[:, gs, :]
    mul_eng.tensor_tensor(dprod_v, t_v, adst_b, OP.mult)
    if do_reduce:
        nc.vector.tensor_reduce(px.dst_col[:, gs], dprod_v, AX.X, OP.add)


def _emit_b2(nc, cpool, pspool, consts, pair, px, do_reduce=False):
    """exp vectors and the c_b broadcast (projections ran inside B1)."""
    ident_sb, sel_sb, w_sb, asrc_sb, adst_sb, bias_sb = consts
    ones_cols = px.hp_big.rearrange("p (c k) -> p c k", k=W)[:, :, O:O + 1]
    nc.vector.memset(ones_cols, 1.0)

    src_col, dst_col = px.src_col, px.dst_col
    if do_reduce:
        nc.vector.tensor_reduce(
            src_col[:], px.prod.rearrange("p (c k) -> p c k", k=O),
            AX.X, OP.add)
        nc.vector.tensor_reduce(
            dst_col[:], px.dprod.rearrange("p (c k) -> p c k", k=O),
            AX.X, OP.add)
    px.edst = cpool.tile([P, NCH], F32, tag="edst", bufs=2, name=f"edst{pair}")
    px.e02 = cpool.tile([P, NCH], F32, tag="e02", bufs=2, name=f"e02{pair}")
    damp = cpool.tile([P, 1], F32, tag="damp", bufs=2, name=f"damp{pair}")
    nc.vector.memset(damp[:], -LN_DAMP)
    nc.scalar.activation(px.edst[:], dst_col[:], AF.Exp)
    nc.scalar.activation(px.e02[:], dst_col[:], AF.Exp, scale=NEG_SLOPE,
                         bias=damp[:])

    # build c_b = exp(0.8 src - ln64)[q] broadcast over partitions with no
    # DMA: PE-transpose src_col into q-major rows, exp row-wise on ACT
    # (cheap: only 16x128 elements), then 16 rank-1 bf16 selector matmuls
    # broadcast the rows into PSUM; plain copies move them out.
    ps_srcT = pspool.tile([NCH, P], F32, tag="hp", bufs=2, name=f"srcT{pair}")
    nc.tensor.transpose(ps_srcT[:], src_col[:], ident_sb[:])
    cbrow = cpool.tile([NCH, P], BF16, tag="cbrow", bufs=2, name=f"cbrow{pair}")
    nc.scalar.activation(cbrow[:], ps_srcT[:], AF.Exp, scale=1.0 - NEG_SLOPE,
                         bias=damp[:NCH])
    px.c_b = cpool.tile([P, N], BF16, tag="c_b", bufs=2, name=f"cb{pair}")
    for piece in range(4):
        ps_bc = pspool.tile([P, 512], F32, tag="hp", bufs=2,
                            name=f"psbc{pair}_{piece}")
        for c4 in range(4):
            c = piece * 4 + c4
            nc.tensor.matmul(
                ps_bc[:, c4 * P:(c4 + 1) * P], sel_sb[:, c * P:(c + 1) * P],
                cbrow[:], start=True, stop=True,
            )
        nc.scalar.copy(px.c_b[:, piece * 512:(piece + 1) * 512], ps_bc[:])


def _emit_b3(nc, epool, pspool, adj_aps, pair, px):
    """scores + output matmul accumulation over m-chunks.

    Per chunk: u = (c_b * edst[m]) max e02[m]   (DVE dual-op ts, 4x mode)
               u = min(u, adj255)               (DVE 2x, or Pool for POOL_MC)
    Pool-chunk matmuls are DEFERRED to the end of the accumulation."""
    psum_out = [
        pspool.tile([W, N // 2], F32, tag="big", bufs=2, name=f"psout{pair}_{h}")
        for h in range(2)
    ]
    us = {}
    for mc in range(NCH):
        adj_ap = adj_aps[mc]
        is_pool = mc in POOL_MC
        u = epool.tile([P, N], BF16, tag="up" if is_pool else "u",
                       bufs=len(POOL_MC) if is_pool else 8,
                       name=f"u{pair}_{mc}")
        us[mc] = u
        nc.vector.tensor_scalar(
            u[:], px.c_b[:], px.edst[:, mc:mc + 1], px.e02[:, mc:mc + 1],
            OP.mult, OP.max,
        )
        mk_eng = nc.gpsimd if is_pool else nc.vector
        mk_eng.tensor_tensor(u[:], u[:], adj_ap, OP.mult)
    for i, mc in enumerate(B3_SEQ):
        for j in range(4):
            nc.tensor.matmul(
                psum_out[j // 2][:, (j % 2) * 512:(j % 2 + 1) * 512],
                px.hp_big[:, mc * W:(mc + 1) * W],
                us[mc][:, j * 512:(j + 1) * 512],
                start=(i == 0),
                stop=(i == len(B3_SEQ) - 1),
                skip_group_check=True,
            )
    return psum_out


def _emit_b4(nc, cpool, pspool, ident_sb, pair, psum_out, out_d):
    """transpose back with normalization fused into the psum drain:
    after the PE transpose, q sits on partitions, so the denominator is a
    per-partition column of the transposed psum -- reciprocal it directly
    and drain psum -> out_sb with ONE ACT copy-with-scale per chunk."""
    outT_sb = cpool.tile([W, N], F32, tag="outT", bufs=1, name=f"outT{pair}")
    rec = cpool.tile([P, NCH], F32, tag="rec", bufs=2, name=f"rec{pair}")
    out_sb = cpool.tile([P, NCH * O], F32, tag="out_sb", bufs=1, name=f"outsb{pair}")
    GRP = 4
    for qg in range(NCH // GRP):
        nc.scalar.copy(outT_sb[:, qg * 512:(qg + 1) * 512],
                       psum_out[qg // 2][:, (qg % 2) * 512:(qg % 2 + 1) * 512])
        psum_tg = pspool.tile([P, GRP * W], F32, tag="hp", bufs=2,
                              name=f"pst{pair}_{qg}")
        for i, qc in enumerate(range(qg * GRP, (qg + 1) * GRP)):
            nc.tensor.transpose(
                psum_tg[:, i * W:(i + 1) * W],
                outT_sb[:, qc * P:(qc + 1) * P], ident_sb[:W, :W]
            )
        den = psum_tg.rearrange("p (c k) -> p c k", k=W)[:, :, O:O + 1]
        nc.vector.reciprocal(
            rec.rearrange("p (c k) -> p c k", k=1)[:, qg * GRP:(qg + 1) * GRP, :],
            den,
        )
        for i, qc in enumerate(range(qg * GRP, (qg + 1) * GRP)):
            if pair == 0:
                # overlaps B3p1 where the DVE is the bottleneck: use ACT
                nc.scalar.activation(
                    out_sb[:, qc * O:(qc + 1) * O], psum_tg[:, i * W:i * W + O],
                    AF.Copy, scale=rec[:, qc:qc + 1],
                )
            else:
                # kernel tail: recip and scale on the same (idle) DVE saves
                # a cross-engine semaphore hop per group
                nc.vector.tensor_scalar(
                    out_sb[:, qc * O:(qc + 1) * O], psum_tg[:, i * W:i * W + O],
                    rec[:, qc:qc + 1], None, OP.mult,
                )
        nc.sync.dma_start(
            out_d[pair].rearrange("(c p) o -> p c o", p=P)[:, qg * GRP:(qg + 1) * GRP, :],
            out_sb.rearrange("p (c k) -> p c k", k=O)[:, qg * GRP:(qg + 1) * GRP, :],
        )


def build_program(reps=1, loop_trip=None):
    nc = bacc.Bacc(
        "TRN2",
        target_bir_lowering=False,
        debug=False,
        enable_asserts=True,
        num_devices=1,
    )
    ht_d = nc.dram_tensor("ht", [F, N], BF16, kind="ExternalInput").ap()
    adjbf_d = nc.dram_tensor("adjbf", [N, N], BF16, kind="ExternalInput").ap()
    w_d = nc.dram_tensor("w", [2, F, O], BF16, kind="ExternalInput").ap()
    asrcb_d = nc.dram_tensor("asrcb", [2, P, O], F32, kind="ExternalInput").ap()
    adstb_d = nc.dram_tensor("adstb", [2, P, O], F32, kind="ExternalInput").ap()
    biasb_d = nc.dram_tensor("biasb", [P, O], F32, kind="ExternalInput").ap()
    ident_d = nc.dram_tensor("ident", [P, P], F32, kind="ExternalInput").ap()
    sel_d = nc.dram_tensor("sel", [NCH, N], BF16, kind="ExternalInput").ap()
    out_d = nc.dram_tensor("out", [2, N, O], F32, kind="ExternalOutput").ap()
    scratch_h = nc.dram_tensor("scratch", [2 * N], F32)

    with tile.TileContext(nc) as tc, ExitStack() as ctx:
        consts_pool = ctx.enter_context(tc.tile_pool(name="consts", bufs=1))
        hpool = ctx.enter_context(tc.tile_pool(name="hpool", bufs=1))
        cpool = ctx.enter_context(tc.tile_pool(name="cpool", bufs=1))
        epool = ctx.enter_context(tc.tile_pool(name="epool", bufs=1))
        pspool = ctx.enter_context(tc.tile_pool(name="psum", bufs=1, space="PSUM"))

        # --- priority DMAs: identity/w/bias + h first (everything hangs off
        # them), then the a-vectors, then the big adjacency slabs.
        w_sb = consts_pool.tile([P, 2 * 2 * O], BF16, tag="w")
        nc.sync.dma_start(
            w_sb.rearrange("k (h c o) -> k h c o", h=2, c=2),
            w_d.rearrange("h (c k) o -> k h c o", k=P),
        )

        # dispatch order (HWDGE serializes at ~0.6us each): h piece 0
        # first (gates B1), then the a-vectors (gate the Pool projection
        # chain), then the rest of h and the small consts.
        hT = [
            hpool.tile([P, N], BF16, tag=f"hT{fc}", name=f"hT{fc}")
            for fc in range(2)
        ]
        h_dmas = []

        def _h_piece(piece):
            for fc in range(2):
                h_dmas.append(nc.sync.dma_start(
                    hT[fc][:, piece * 512:(piece + 1) * 512],
                    ht_d[fc * P:(fc + 1) * P, piece * 512:(piece + 1) * 512],
                ))

        _h_piece(0)
        asrc_sb = consts_pool.tile([P, 2 * O], F32, tag="asrc")
        adst_sb = consts_pool.tile([P, 2 * O], F32, tag="adst")
        av_dmas = [
            nc.sync.dma_start(
                asrc_sb.rearrange("p (h o) -> p h o", h=2),
                asrcb_d.rearrange("h p o -> p h o"),
            ),
            nc.sync.dma_start(
                adst_sb.rearrange("p (h o) -> p h o", h=2),
                adstb_d.rearrange("h p o -> p h o"),
            ),
        ]
        for piece in range(1, 4):
            _h_piece(piece)
        bias_sb = consts_pool.tile([P, O], F32, tag="bias")
        nc.sync.dma_start(bias_sb[:], biasb_d[:])
        ident_sb = consts_pool.tile([P, P], F32, tag="ident")
        nc.sync.dma_start(ident_sb[:], ident_d[:])

        sel_sb = consts_pool.tile([NCH, N], BF16, tag="sel")
        nc.sync.dma_start(sel_sb[:], sel_d[:])

        if loop_trip is not None:
            from concourse.engine_type import EngineType
            _loop_cm = tc.For_i(
                0, loop_trip, 1,
                hint_engines=(EngineType.PE, EngineType.DVE,
                              EngineType.Activation, EngineType.SP,
                              EngineType.Pool),
            )
            _loop_cm.__enter__()
        for rep in range(reps):
          # adjacency: independent 2-chunk bf16 {0,1} slabs (host-prepped,
          # no cast DMA), dispatched on SP/HWDGE.  adj_aps maps chunk -> AP.
          adj_aps = {}
          adj_r = adjbf_d.rearrange("(c p) q -> p c q", p=P)
          for i in range(0, NCH, 2):
              slab = hpool.tile([P, 2 * N], BF16, tag=f"adj{i}",
                                name=f"adj{i}")
              adj_dma = nc.sync.dma_start(
                  slab.rearrange("p (c q) -> p c q", q=N),
                  adj_r[:, i:i + 2, :],
              )
              adj_aps[i] = slab[:, 0:N]
              adj_aps[i + 1] = slab[:, N:2 * N]
              # don't compete with the latency-critical h load for DMA
              # bandwidth; the first B3 use of adj is ~10us in.
              add_dep_helper(adj_dma.ins, h_dmas[-1].ins,
                             reason="delay adj behind h")

          consts = (ident_sb, sel_sb, w_sb, asrc_sb, adst_sb, bias_sb)
          px = [_emit_b1(nc, cpool, pspool, consts, hT, pair) for pair in range(2)]
          # emission order keeps every in-order engine queue stall-free:
          # B1p0 -> B1p1 -> B2p0 -> B2p1 -> B3p0 -> B4p0 -> B3p1 -> B4p1.
          # pair 1's reduces execute on the DVE while it waits for pair 0's
          # c_b (they are ready first), so the whole c_b-p1 chain finishes
          # on PE/ACT during B3p0 and B3p1 starts with no gap.
          for g in range(NCH // GB1):
              _emit_b1_group(nc, pspool, consts, hT, 0, px[0], g, True)
          for g in range(NCH // GB1):
              _emit_b1_group(nc, pspool, consts, hT, 1, px[1], g, True)
          _emit_b2(nc, cpool, pspool, consts, 0, px[0])
          _emit_b2(nc, cpool, pspool, consts, 1, px[1])
          psum_out0 = _emit_b3(nc, epool, pspool, adj_aps, 0, px[0])
          _emit_b4(nc, cpool, pspool, ident_sb, 0, psum_out0, out_d)
          psum_out1 = _emit_b3(nc, epool, pspool, adj_aps, 1, px[1])
          _emit_b4(nc, cpool, pspool, ident_sb, 1, psum_out1, out_d)

        if loop_trip is not None:
            _loop_cm.__exit__(None, None, None)

    nc.compile()
    return nc


_CACHED = {}


def _get_program(zero_bias=True):
    global ZERO_BIAS
    key = ("nc", zero_bias)
    if key not in _CACHED:
        ZERO_BIAS = zero_bias
        _CACHED[key] = build_program()
    return _CACHED[key]


def make_in_maps(h, adj, w, a_src, a_dst, bias):
    h = np.ascontiguousarray(np.asarray(h, dtype=np.float32))
    adj = np.asarray(adj)
    w = np.asarray(w, dtype=np.float32)
    a_src = np.asarray(a_src, dtype=np.float32).reshape(4, O)
    a_dst = np.asarray(a_dst, dtype=np.float32).reshape(4, O)
    bias = np.asarray(bias, dtype=np.float32).reshape(O)

    adjT = np.ascontiguousarray(adj.transpose(0, 2, 1))
    biasb = np.ascontiguousarray(np.broadcast_to(bias, (P, O)))
    ident = np.eye(P, dtype=np.float32)
    sel = np.kron(np.eye(NCH, dtype=np.float32),
                  np.ones((1, P), np.float32)).astype(ml_dtypes.bfloat16)

    in_maps = []
    for c in range(N_CORES):
        b = c // 2
        hs = [2 * (c % 2), 2 * (c % 2) + 1]
        asrcb = np.ascontiguousarray(
            np.broadcast_to(a_src[hs][:, None, :], (2, P, O)))
        adstb = np.ascontiguousarray(
            np.broadcast_to(a_dst[hs][:, None, :], (2, P, O)))
        in_maps.append({
            "ht": np.ascontiguousarray(h[b].T).astype(ml_dtypes.bfloat16),
            "adjbf": adjT[b].astype(ml_dtypes.bfloat16),
            "w": np.ascontiguousarray(w[hs]).astype(ml_dtypes.bfloat16),
            "asrcb": asrcb,
            "adstb": adstb,
            "biasb": biasb,
            "ident": ident,
            "sel": sel,
        })
    return in_maps


def assemble(results):
    out = np.empty((4, 4, N, O), dtype=np.float32)
    for c in range(N_CORES):
        b = c // 2
        for i, hd in enumerate((2 * (c % 2), 2 * (c % 2) + 1)):
            out[b, hd] = results[c]["out"][i]
    return out


def kernel(h, adj, w, a_src, a_dst, bias):
    nc = _get_program(zero_bias=not np.any(np.asarray(bias)))
    in_maps = make_in_maps(h, adj, w, a_src, a_dst, bias)
    res = run_bass_kernel_spmd(nc, in_maps, core_ids=list(range(N_CORES)))
    return assemble(res.results)
